# revision 1
# baseline (speedup 1.0000x reference)
"""CrossNet (keras CrossLayer x3) Trainium2 kernel.

Math: for x0 = x[:, 0, :] (B, D), the layer recurrence
    cross_{i+1} = sum(cross_i) * (W_i * x0) + b_i + x0
collapses to per-row scalars only:
    s0 = sum_j x0[j],  t_i = dot(x0, W_i),  beta_i = sum_j b_i[j]
    s1 = s0*t0 + beta0 + s0
    s2 = s1*t1 + beta1 + s0
    out = s2 * (W2 * x0) + b2 + x0  =  x0 * (s2*W2 + 1) + b2

Per-core kernel (batch sharded 8 ways):
  - DMA 1 MiB x-tiles (1024 rows, 8 rows/partition)
  - PE transpose 128x128 chunks -> PSUM, copy to SBUF, tiny matmuls against
    M = [ones, W0, W1] (256x3) accumulate per-row dots into PSUM
  - small DVE ops for the s2 recurrence
  - V = s2*W2b + 1 via tensor_scalar, out = x * V, DMA out
"""

from contextlib import ExitStack

import numpy as np

B, D, L = 262144, 256, 3
N_CORES = 8
ROWS = B // N_CORES          # rows per core = 32768
TILE_ROWS = 1024             # rows per SBUF tile (1 MiB)
G = TILE_ROWS // 128         # row-groups per tile = 8
N_TILES = ROWS // TILE_ROWS  # 32

_cache: dict = {}


def _build(beta0: float, beta1: float):
    import concourse.bacc as bacc
    import concourse.tile as tile
    from concourse import mybir
    from concourse.masks import make_identity

    f32 = mybir.dt.float32
    op_add = mybir.AluOpType.add
    op_mult = mybir.AluOpType.mult

    nc = bacc.Bacc("TRN2", target_bir_lowering=False, debug=False,
                   num_devices=N_CORES)
    x_d = nc.dram_tensor("x", [ROWS, D], f32, kind="ExternalInput").ap()
    m_d = nc.dram_tensor("m", [D, 3], f32, kind="ExternalInput").ap()
    w2_d = nc.dram_tensor("w2", [128, D], f32, kind="ExternalInput").ap()
    o_d = nc.dram_tensor("out", [ROWS, D], f32, kind="ExternalOutput").ap()

    # row -> (tile t, partition p, group g): row = t*1024 + p*8 + g
    x_r = x_d.rearrange("(t p g) d -> t p (g d)", p=128, g=G)
    o_r = o_d.rearrange("(t p g) d -> t p (g d)", p=128, g=G)

    with tile.TileContext(nc) as tc, ExitStack() as ctx:
        consts = ctx.enter_context(tc.tile_pool(name="consts", bufs=1))
        xp = ctx.enter_context(tc.tile_pool(name="xp", bufs=3))
        vp = ctx.enter_context(tc.tile_pool(name="vp", bufs=2))
        op_ = ctx.enter_context(tc.tile_pool(name="op", bufs=3))
        xtp = ctx.enter_context(tc.tile_pool(name="xtp", bufs=4))
        ptp = ctx.enter_context(tc.tile_pool(name="ptp", bufs=4, space="PSUM"))
        pdp = ctx.enter_context(tc.tile_pool(name="pdp", bufs=2, space="PSUM"))
        sm = ctx.enter_context(tc.tile_pool(name="sm", bufs=4))

        identity = consts.tile([128, 128], f32)
        make_identity(nc, identity)
        m_sb = consts.tile([128, 2, 3], f32)
        nc.sync.dma_start(out=m_sb[:, 0, :], in_=m_d[0:128, :])
        nc.sync.dma_start(out=m_sb[:, 1, :], in_=m_d[128:256, :])
        w2b = consts.tile([128, D], f32)
        nc.sync.dma_start(out=w2b, in_=w2_d)

        use_b = beta0 != 0.0 or beta1 != 0.0

        for t in range(N_TILES):
            x_t = xp.tile([128, G, D], f32)
            nc.sync.dma_start(out=x_t, in_=x_r[t])

            dots = pdp.tile([128, G, 3], f32)
            for g in range(G):
                for h in range(2):
                    tp = ptp.tile([128, 128], f32)
                    nc.tensor.transpose(tp, x_t[:, g, 128 * h:128 * (h + 1)],
                                        identity)
                    xt = xtp.tile([128, 128], f32)
                    if (g * 2 + h) % 2 == 0:
                        nc.scalar.copy(xt, tp)
                    else:
                        nc.vector.tensor_copy(xt, tp)
                    nc.tensor.matmul(dots[:, g, :], lhsT=xt,
                                     rhs=m_sb[:, h, :],
                                     start=(h == 0), stop=(h == 1))

            # s2 = s0*((t0+1)*t1 + 1) [+ beta0*t1 + beta1]
            s0 = dots[:, :, 0]
            t0 = dots[:, :, 1]
            t1 = dots[:, :, 2]
            a = sm.tile([128, G], f32, tag="a")
            nc.vector.tensor_scalar(out=a, in0=t0, scalar1=1.0, scalar2=None,
                                    op0=op_add)
            b_ = sm.tile([128, G], f32, tag="b")
            nc.vector.tensor_tensor(out=b_, in0=a, in1=t1, op=op_mult)
            c_ = sm.tile([128, G], f32, tag="c")
            nc.vector.tensor_scalar(out=c_, in0=b_, scalar1=1.0, scalar2=None,
                                    op0=op_add)
            s2 = sm.tile([128, G], f32, tag="s2")
            nc.vector.tensor_tensor(out=s2, in0=c_, in1=s0, op=op_mult)
            if use_b:
                e_ = sm.tile([128, G], f32, tag="e")
                nc.vector.tensor_scalar(out=e_, in0=t1, scalar1=beta0,
                                        scalar2=beta1, op0=op_mult, op1=op_add)
                nc.vector.tensor_tensor(out=s2, in0=s2, in1=e_, op=op_add)

            v_t = vp.tile([128, G, D], f32)
            for g in range(G):
                nc.vector.tensor_scalar(out=v_t[:, g, :], in0=w2b,
                                        scalar1=s2[:, g:g + 1], scalar2=1.0,
                                        op0=op_mult, op1=op_add)

            o_t = op_.tile([128, G, D], f32)
            nc.gpsimd.tensor_tensor(out=o_t, in0=x_t, in1=v_t, op=op_mult)
            nc.sync.dma_start(out=o_r[t], in_=o_t)

    nc.compile()
    return nc


def kernel(x: np.ndarray, W: np.ndarray, b: np.ndarray) -> np.ndarray:
    from concourse.bass_utils import run_bass_kernel_spmd

    x = np.asarray(x, dtype=np.float32).reshape(B, D)
    W = np.asarray(W, dtype=np.float32)
    b = np.asarray(b, dtype=np.float32)

    m = np.stack([np.ones(D, np.float32), W[0], W[1]], axis=1)  # (D, 3)
    w2b = np.broadcast_to(W[2], (128, D)).copy()                # replicated
    beta0 = float(b[0].sum())
    beta1 = float(b[1].sum())

    key = (beta0, beta1)
    if key not in _cache:
        _cache[key] = _build(beta0, beta1)
    nc = _cache[key]

    in_maps = [
        {"x": x[c * ROWS:(c + 1) * ROWS], "m": m, "w2": w2b}
        for c in range(N_CORES)
    ]
    res = run_bass_kernel_spmd(nc, in_maps, core_ids=list(range(N_CORES)))
    out = np.concatenate([r["out"] for r in res.results], axis=0)
    if np.any(b[2]):
        out = out + b[2]
    return out
